# revision 6
# baseline (speedup 1.0000x reference)
"""Trainium2 Bass kernel for nn_LorentzGraphConvolution (v7).

Row-sharded across 8 NeuronCores: core c owns rows [c*1536, (c+1)*1536) of
the attention matrix / output. Every core redundantly computes the tiny
linear phase (h, k for all N; q for its local rows) from broadcast inputs,
so no collectives are needed.

v7 structure: the linear phase is produced in j-THIRDS (32 tiles) and
chunks 0 and 1 of phase C are consumed J-MAJOR, interleaved with the
linear thirds (linear third t gates octet t*4 of every chunk, so two
chunks' worth of attention work hides the whole linear phase).  Chunk 2
runs pure afterwards with a deeper PSUM pipeline.

PSUM partitioning (8 banks):
  interleave scope: psL 2x[128,512] (2) + psA 2x[128,1024] (4)
                    + supT_c0 + supT_c1 (2) = 8
  chunk-2 scope:    psA2 3x[128,1024] (6) + psS2 supT+scratch (2) = 8

The linear phase's PSUM->SBUF move is fused with the Lorentz
normalization multiply on the DVE (no ACT Copy pass); ACT does the
Square (spatial sq-sums) and the tiny slab sigmoids only, keeping ACT
free for phase C's sigmoid wall (the roofline of this kernel).

Adjacency is pre-transposed AND pre-cast to BF16 on the host: the DVE
multiplicative mask runs in 2x perf mode on bf16 (~690ns/pair vs ~994
for fp8), and the PE additive-mask matmul (1 pair in PE_EVERY) is the
same cost in bf16 as fp8.
"""

import math
import os
import sys
from contextlib import ExitStack

for _p in ("/opt/trn_rl_repo", "/root/.axon_site/_ro/trn_rl_repo", "/root/.axon_site"):
    if os.path.isdir(_p) and _p not in sys.path:
        sys.path.insert(0, _p)

import ml_dtypes
import numpy as np

import concourse.bass as bass
import concourse.tile as tile
from concourse import bacc, bass_utils, masks, mybir
from concourse.tile import add_dep_helper

DT = mybir.dt
F32 = DT.float32
BF16 = DT.bfloat16
I32 = DT.int32
AF = mybir.ActivationFunctionType
ALU = mybir.AluOpType
AXX = mybir.AxisListType.X

N_FULL = 12288
D = 64
N_CORES = 8
R_FULL = N_FULL // N_CORES  # 1536 rows per core

PE_EVERY = 3  # 1 of every PE_EVERY pairs uses the PE additive mask


def pick_big(sig_scale):
    """Smallest bf16-exact value >= 45/sig_scale (pushes masked logits
    below sigmoid(-24) while staying exactly representable)."""
    want = 45.0 / sig_scale
    v = float(np.float32(ml_dtypes.bfloat16(want)))
    while v < want:
        want *= 1.01
        v = float(np.float32(ml_dtypes.bfloat16(want)))
    return v


def emit(tc, io, nn, rr, esc, esc_q, esc_k, sig_scale, sig_bias, big):
    nc = tc.nc
    TJ = nn // 128          # 96 global j tiles
    TL = rr // 128          # 12 local i tiles
    NCH = 3                 # i-chunks per core
    IC = rr // NCH          # 512 rows per chunk
    NOCT = TJ // 8          # 12 octets of j tiles per chunk
    TH = TJ // 3            # 32 j tiles per linear third
    assert IC == 512 and TJ % 8 == 0 and TH == 32

    ctx = ExitStack()
    const = ctx.enter_context(tc.tile_pool(name="const", bufs=1))
    persist = ctx.enter_context(tc.tile_pool(name="persist", bufs=1))
    xp = ctx.enter_context(tc.tile_pool(name="xp", bufs=1))
    htp = ctx.enter_context(tc.tile_pool(name="htp", bufs=1))
    kdp = ctx.enter_context(tc.tile_pool(name="kdp", bufs=1))
    oneshot = ctx.enter_context(tc.tile_pool(name="oneshot", bufs=1))
    slab = ctx.enter_context(tc.tile_pool(name="slab", bufs=2))
    sqfp = ctx.enter_context(tc.tile_pool(name="sqfp", bufs=3))
    small = ctx.enter_context(tc.tile_pool(name="small", bufs=4))
    oct_pool = ctx.enter_context(tc.tile_pool(name="octs", bufs=5))
    sig_pool = ctx.enter_context(tc.tile_pool(name="sig", bufs=4))
    out_pool = ctx.enter_context(tc.tile_pool(name="outp", bufs=4))

    # ---- small inputs first (critical path: xqT -> hq -> qm) ----------
    xqT_s = const.tile([65, rr], BF16)
    nc.sync.dma_start(xqT_s[:], io["xqT"][:])
    wT_s = const.tile([65, 64], BF16)
    nc.sync.dma_start(wT_s[:], io["wT"][:])
    wqT_s = const.tile([65, 64], BF16)
    nc.sync.dma_start(wqT_s[:], io["wqT"][:])
    wkT_s = const.tile([65, 64], BF16)
    nc.sync.dma_start(wkT_s[:], io["wkT"][:])
    bigI = const.tile([128, 128], BF16)
    nc.sync.dma_start(bigI[:], io["bigi"][:])
    xT_s = xp.tile([65, nn], BF16)
    NXS = 6
    xdmas = []
    for xs in range(NXS):
        w0 = xs * (nn // NXS)
        xdmas.append(nc.sync.dma_start(xT_s[:, w0:w0 + nn // NXS],
                                       io["xT"][:, w0:w0 + nn // NXS]))
    adj_gate = xdmas[1]

    ident = const.tile([64, 64], F32)
    masks.make_identity(nc, ident[:])
    sig_bias_big = const.tile([128, 1], F32)
    nc.vector.memset(sig_bias_big[:], sig_bias - big * sig_scale)
    sig_bias_ap = const.tile([128, 1], F32)
    nc.vector.memset(sig_bias_ap[:], sig_bias)
    magic = const.tile([128, 1], I32)
    nc.vector.memset(magic[:], 0x5F3759DF)

    # persistent per-core tensors. "pad" slabs put tile t's 64 features in
    # cols [t*128, t*128+64) so a 128x128 block DMA-transpose lands the
    # features at partitions 0:64; col 64 holds the bias-ones row.
    hpad = persist.tile([128, TJ * 128], BF16)
    hpad3 = hpad.rearrange("p (t c) -> p t c", c=128)
    hT_flat = htp.tile([128, TJ * 128], BF16)
    hT3 = hT_flat.rearrange("p (t n) -> p t n", n=128)
    kdense = kdp.tile([128, TJ * 64], BF16)
    kdense3 = kdense.rearrange("p (t d) -> p t d", d=64)
    # k^T stacked pairs: block t' rows 0:64 = kT[2t'], rows 64:128 = kT[2t'+1]
    kT_stk = persist.tile([128, (TJ // 2) * 128], BF16)
    kT3 = kT_stk.rearrange("p (t n) -> p t n", n=128)
    # qm^T duplicated in both partition halves for the row-packed MM1 pairs
    qmT_full = persist.tile([128, TL * 128], BF16)

    adjt2 = io["adjt"]

    # =========== scope 1: linear + chunks 0,1 j-major =================
    ps1 = ExitStack()
    psL = ps1.enter_context(tc.tile_pool(name="psL", bufs=2, space="PSUM"))
    psA = ps1.enter_context(tc.tile_pool(name="psA", bufs=2, space="PSUM"))
    psS = ps1.enter_context(tc.tile_pool(name="psS", bufs=1, space="PSUM"))

    # =========== linear building block ================================
    def lin_group(psp, gs, sb, g0, lhsT_fn, rhs_w, a_time, c0_time, dest3,
                  ones_col):
        """Produce dest3[:, g0:g0+gs, 0:64] = Lorentz-normalized linear
        output for `gs` tiles (col 0 = time, 1:64 scaled spatial), with
        the PSUM->SBUF move fused into the normalization multiply."""
        ngrp = gs // sb
        tot = slab.tile([128, 16], F32, tag="tot", name="tot")[:, :gs]
        logit = slab.tile([128, 16], F32, tag="lgt", name="lgt")[:, :gs]
        ps_list = []
        for b in range(ngrp):
            ps = psp.tile([128, 512], F32, tag="linps", name="linps")
            psv = ps[:, : sb * 64]
            for u in range(sb):
                nc.tensor.matmul(psv[:, u * 64:(u + 1) * 64],
                                 lhsT_fn(g0 + b * sb + u), rhs_w,
                                 start=True, stop=True)
            sqf = sqfp.tile([128, 512], BF16, tag="sqf", name="sqf")
            sqv = sqf[:, : sb * 64]
            nc.scalar.activation(sqv, psv, AF.Square)
            sq3 = sqv.rearrange("p (t d) -> p t d", d=64)
            nc.vector.tensor_reduce(tot[:, b * sb:(b + 1) * sb],
                                    sq3[:, :, 1:64], axis=AXX, op=ALU.add)
            ps3 = psv.rearrange("p (t d) -> p t d", d=64)
            nc.vector.tensor_copy(logit[:, b * sb:(b + 1) * sb],
                                  ps3[:, :, 0])
            ps_list.append(ps3)
        # slab-level Lorentz normalization for the group
        sg = slab.tile([128, 16], F32, tag="sg", name="sg")[:, :gs]
        nc.scalar.activation(sg, logit, AF.Sigmoid)
        time = slab.tile([128, 16], F32, tag="tm", name="tm")[:, :gs]
        nc.vector.tensor_scalar(time, sg, a_time, c0_time, ALU.mult, ALU.add)
        t2 = slab.tile([128, 16], F32, tag="t2", name="t2")[:, :gs]
        nc.vector.tensor_tensor(t2, time, time, ALU.mult)
        sqc = slab.tile([128, 16], F32, tag="sc", name="sc")[:, :gs]
        nc.vector.tensor_scalar_max(sqc, tot, 1e-8)
        rec = slab.tile([128, 16], F32, tag="rc", name="rc")[:, :gs]
        nc.vector.reciprocal(rec, sqc)
        ratio = slab.tile([128, 16], F32, tag="ra", name="ra")[:, :gs]
        nc.vector.scalar_tensor_tensor(ratio, t2, -1.0, rec,
                                       ALU.add, ALU.mult)
        # rsqrt(ratio) via bit trick + 1 Newton iteration (DVE only)
        sh = slab.tile([128, 16], I32, tag="sh", name="sh")[:, :gs]
        nc.vector.tensor_scalar(sh, ratio.bitcast(I32), 1, None,
                                ALU.arith_shift_right)
        y0 = slab.tile([128, 16], F32, tag="y0", name="y0")[:, :gs]
        nc.vector.tensor_tensor(y0.bitcast(I32),
                                magic[:].to_broadcast((128, gs)), sh,
                                ALU.subtract)
        ysq = slab.tile([128, 16], F32, tag="yq", name="yq")[:, :gs]
        nc.vector.tensor_tensor(ysq, y0, y0, ALU.mult)
        tq = slab.tile([128, 16], F32, tag="tq", name="tq")[:, :gs]
        nc.vector.tensor_tensor(tq, ysq, ratio, ALU.mult)
        w = slab.tile([128, 16], F32, tag="w", name="w")[:, :gs]
        nc.vector.tensor_scalar(w, tq, -0.5, 1.5, ALU.mult, ALU.add)
        rsq = slab.tile([128, 16], F32, tag="rq", name="rq")[:, :gs]
        nc.vector.tensor_tensor(rsq, y0, w, ALU.mult)
        sqs = slab.tile([128, 16], F32, tag="ss", name="ss")[:, :gs]
        nc.vector.tensor_tensor(sqs, ratio, rsq, ALU.mult)
        # fused PSUM->SBUF normalization multiply, then time / ones cols
        for b in range(ngrp):
            nc.vector.tensor_tensor(
                dest3[:, g0 + b * sb:g0 + (b + 1) * sb, 0:64], ps_list[b],
                sqs[:, b * sb:(b + 1) * sb].to_broadcast((128, sb, 64)),
                ALU.mult)
        nc.vector.tensor_copy(dest3[:, g0:g0 + gs, 0], time)
        if ones_col:
            nc.vector.memset(dest3[:, g0:g0 + gs, 64], 1.0)

    # =========== phase C machinery ====================================
    class Chunk:
        def __init__(self, c, pool):
            self.c = c
            self.supT = pool.tile([128, 512], F32, tag="supT%d" % c,
                                  name="supT")
            self.qch = qmT_full[:, c * IC:(c + 1) * IC]
            self.pending = None
            self.prev_lo = None
            self.prev_hi = None
            self.seq = 0

    def emit_mm2(st, stop):
        sig_t, jl = st.pending
        start = jl == 0
        sA = nc.tensor.matmul(st.supT[0:64, :], hpad3[:, jl, 0:64],
                              sig_t[:, 0:512], start=start, stop=stop,
                              tile_position=(0, 0))
        if st.prev_lo is not None:
            add_dep_helper(sA.ins, st.prev_lo.ins, sync=False,
                           reason="supT lo accum order")
        st.prev_lo = sA
        sB = nc.tensor.matmul(st.supT[64:128, :], hpad3[:, jl + 1, 0:64],
                              sig_t[:, 512:1024], start=start, stop=stop,
                              tile_position=(0, 64))
        if st.prev_hi is not None:
            add_dep_helper(sB.ins, st.prev_hi.ins, sync=False,
                           reason="supT hi accum order")
        st.prev_hi = sB

    first_oct = [True]

    def emit_octet(st, o, att_pool):
        oct = oct_pool.tile([128, 8 * 512], BF16, tag="oct", name="oct")
        oct3 = oct.rearrange("p (t q) -> p t q", q=512)
        r0 = (st.c * NOCT + o) * 128
        odma = nc.gpsimd.dma_start(oct[:], adjt2[r0:r0 + 128, :])
        if first_oct[0]:
            first_oct[0] = False
            add_dep_helper(odma.ins, adj_gate.ins, sync=True,
                           reason="inputs before adj prefetch")
        for pr in range(4):
            jl = o * 8 + pr * 2
            tp = jl // 2
            pe_mask = st.seq % PE_EVERY == 0
            st.seq += 1
            attT = att_pool.tile([128, 1024], F32, tag="attT", name="attT")
            mmA = nc.tensor.matmul(attT[:, 0:512],
                                   kT_stk[0:64, tp * 128:(tp + 1) * 128],
                                   st.qch[0:64, :], start=True,
                                   stop=not pe_mask, tile_position=(0, 0))
            mmB = nc.tensor.matmul(attT[:, 512:1024],
                                   kT_stk[64:128, tp * 128:(tp + 1) * 128],
                                   st.qch[64:128, :], start=True,
                                   stop=not pe_mask, tile_position=(64, 0))
            if pe_mask:
                # additive mask on PE: attT += BIG * adjT
                mA = nc.tensor.matmul(attT[:, 0:512], bigI[:],
                                      oct3[:, 2 * pr, :], start=False,
                                      stop=True)
                add_dep_helper(mA.ins, mmA.ins, sync=False,
                               reason="mask after ip A")
                mB = nc.tensor.matmul(attT[:, 512:1024], bigI[:],
                                      oct3[:, 2 * pr + 1, :],
                                      start=False, stop=True)
                add_dep_helper(mB.ins, mmB.ins, sync=False,
                               reason="mask after ip B")
            sig_t = sig_pool.tile([128, 1024], BF16, tag="sig",
                                  name="sig_t")
            nc.scalar.activation(
                sig_t[:], attT[:], AF.Sigmoid,
                bias=sig_bias_big[:] if pe_mask else sig_bias_ap[:],
                scale=sig_scale)
            if pe_mask:
                rhs = sig_t[:]
            else:
                # multiplicative mask on DVE (bf16 x bf16 -> 2x mode)
                sm = sig_pool.tile([128, 1024], BF16, tag="sm", name="sm")
                nc.vector.tensor_tensor(
                    sm[:], sig_t[:],
                    oct[:, (2 * pr) * 512:(2 * pr + 2) * 512], ALU.mult)
                rhs = sm[:]
            if st.pending is not None:
                emit_mm2(st, stop=False)
            st.pending = (rhs, jl)

    def fast_rsqrt4(dst, x):
        """[128,4] rsqrt, 2 Newton iterations (output normalization)."""
        xi = x.bitcast(I32)
        sh = small.tile([128, 4], I32, tag="fsh", name="fsh")
        nc.vector.tensor_scalar(sh[:], xi, 1, None, ALU.arith_shift_right)
        y = dst
        nc.vector.tensor_tensor(y.bitcast(I32),
                                magic[:].to_broadcast((128, 4)), sh[:],
                                ALU.subtract)
        for it in range(2):
            ysq = small.tile([128, 4], F32, tag="fyq%d" % it, name="fyq")
            nc.vector.tensor_tensor(ysq[:], y, y, ALU.mult)
            t = small.tile([128, 4], F32, tag="ft%d" % it, name="ft")
            nc.vector.tensor_tensor(t[:], ysq[:], x, ALU.mult)
            w = small.tile([128, 4], F32, tag="fw%d" % it, name="fw")
            nc.vector.tensor_scalar(w[:], t[:], -0.5, 1.5, ALU.mult,
                                    ALU.add)
            yn = small.tile([128, 4], F32, tag="fy%d" % it, name="fy")
            nc.vector.tensor_tensor(yn[:], y, w[:], ALU.mult)
            y = yn[:]
        nc.vector.tensor_copy(dst, y)

    def finish_chunk(st, scratch_pool, scratch_tag):
        emit_mm2(st, stop=True)
        c = st.c
        lo_s = small.tile([64, 512], F32, tag="los", name="lo_s")
        nc.vector.tensor_copy(lo_s[:], st.supT[0:64, :])
        sup_s = small.tile([64, 512], F32, tag="sups", name="sup_s")
        nc.vector.tensor_tensor(sup_s[:], st.supT[64:128, :], lo_s[:],
                                ALU.add)
        sq_all = out_pool.tile([128, 4 * 64], F32, tag="sqall",
                               name="sq_all")
        sq_all3 = sq_all.rearrange("p (s d) -> p s d", d=64)
        o_raw = out_pool.tile([128, 4 * 64], F32, tag="oraw", name="o_raw")
        o_raw3 = o_raw.rearrange("p (s d) -> p s d", d=64)
        for s in range(4):
            supn = scratch_pool.tile([128, 512], F32, tag=scratch_tag,
                                     name="supn")
            supn = supn[:, 0:64]
            nc.tensor.transpose(supn, sup_s[:, s * 128:(s + 1) * 128],
                                ident[:])
            nc.scalar.activation(sq_all3[:, s, :], supn, AF.Square)
            nc.vector.tensor_copy(o_raw3[:, s, :], supn)
        tot4 = small.tile([128, 4], F32, tag="ftot", name="tot4")
        nc.vector.tensor_reduce(tot4[:], sq_all3, axis=AXX, op=ALU.add)
        inner4 = small.tile([128, 4], F32, tag="finn", name="inner4")
        # inner = tot - 2*s0^2  (= -s0^2 + sum_{d>=1} s_d^2)
        nc.vector.scalar_tensor_tensor(inner4[:], sq_all3[:, :, 0], -2.0,
                                       tot4[:], ALU.mult, ALU.add)
        negv = small.tile([128, 4], F32, tag="fneg", name="negv")
        nc.vector.tensor_scalar_mul(negv[:], inner4[:], -1.0)
        absv = small.tile([128, 4], F32, tag="fabs", name="absv")
        nc.vector.tensor_tensor(absv[:], inner4[:], negv[:], ALU.max)
        clip4 = small.tile([128, 4], F32, tag="fclip", name="clip4")
        nc.vector.tensor_scalar_max(clip4[:], absv[:], 1e-8)
        rs4 = small.tile([128, 4], F32, tag="frs", name="rs4")
        fast_rsqrt4(rs4[:], clip4[:])
        o_t = out_pool.tile([128, 4 * 64], F32, tag="otile", name="o_t")
        o_t3 = o_t.rearrange("p (s d) -> p s d", d=64)
        nc.vector.tensor_tensor(o_t3[:], o_raw3[:],
                                rs4[:].to_broadcast((128, 4, 64)),
                                ALU.mult)
        nc.sync.dma_start(
            io["out"][c * IC:(c + 1) * IC, :].rearrange(
                "(s p) d -> p s d", p=128), o_t3[:])

    # =========== schedule =============================================
    def h_group(g):
        lin_group(psL, 16, 8, g * 16,
                  lambda t: xT_s[:, t * 128:(t + 1) * 128],
                  wT_s[:], esc, 1.1, hpad3, True)

    def hT_transpose(th):
        s0 = th * TH
        nc.sync.dma_start(hT3[:, s0:s0 + TH, :],
                          hpad[:, s0 * 128:(s0 + TH) * 128], transpose=True)

    def k_group(g):
        lin_group(psL, 16, 8, g * 16,
                  lambda t: hT_flat[0:65, t * 128:(t + 1) * 128],
                  wkT_s[:], esc_k, 1.1, kdense3, False)

    def kT_transpose(th):
        s0 = th * TH
        nc.sync.dma_start(kT3[:, th * (TH // 2):(th + 1) * (TH // 2), :],
                          kdense[:, s0 * 64:(s0 + TH) * 64], transpose=True)

    # hq (local rows) -> transpose -> qm
    hqpad = oneshot.tile([128, TL * 128], BF16, tag="hq")
    hqpad3 = hqpad.rearrange("p (t c) -> p t c", c=128)
    lin_group(psL, 12, 6, 0, lambda t: xqT_s[:, t * 128:(t + 1) * 128],
              wT_s[:], esc, 1.1, hqpad3, True)
    hqT_flat = oneshot.tile([128, TL * 128], BF16, tag="hqT")
    nc.sync.dma_start(hqT_flat.rearrange("p (t n) -> p t n", n=128),
                      hqpad[:], transpose=True)

    h_group(0)
    h_group(1)

    qm_pad = oneshot.tile([128, TL * 128], BF16, tag="qmpad")
    qm_pad3 = qm_pad.rearrange("p (t c) -> p t c", c=128)
    lin_group(psL, 12, 6, 0, lambda t: hqT_flat[0:65, t * 128:(t + 1) * 128],
              wqT_s[:], -esc_q, -1.1, qm_pad3, False)
    nc.vector.tensor_copy(qm_pad3[:, :, 64:128], qm_pad3[:, :, 0:64])
    nc.sync.dma_start(qmT_full.rearrange("p (t n) -> p t n", n=128),
                      qm_pad[:], transpose=True)

    hT_transpose(0)
    k_group(0)
    k_group(1)
    kT_transpose(0)

    ch0 = Chunk(0, psS)
    ch1 = Chunk(1, psS)

    # j-major interleave: third t's octets for chunks 0+1, with third
    # t+1's linear production woven between them.
    def octet_round(t):
        for o in range(4 * t, 4 * t + 4):
            yield ("oct", ch0, o)
            yield ("oct", ch1, o)

    # third 0 octets + third 1 production
    prod1 = [("h", 2), ("h", 3), ("hT", 1), ("k", 2), ("k", 3), ("kT", 1)]
    prod2 = [("h", 4), ("h", 5), ("hT", 2), ("k", 4), ("k", 5), ("kT", 2)]

    def run_round(t, prod):
        ops = list(octet_round(t))
        # weave production ops between octet emissions
        merged = []
        pi = 0
        for i, op in enumerate(ops):
            merged.append(op)
            while pi < len(prod) and pi <= i:
                merged.append(prod[pi])
                pi += 1
        merged.extend(prod[pi:])
        for op in merged:
            if op[0] == "oct":
                emit_octet(op[1], op[2], psA)
            elif op[0] == "h":
                h_group(op[1])
            elif op[0] == "k":
                k_group(op[1])
            elif op[0] == "hT":
                hT_transpose(op[1])
            elif op[0] == "kT":
                kT_transpose(op[1])

    run_round(0, prod1)
    run_round(1, prod2)
    run_round(2, [])
    finish_chunk(ch0, psL, "linps")
    finish_chunk(ch1, psL, "linps")
    ps1.close()

    # =========== scope 2: chunk 2 pure, deeper psum pipeline ==========
    ps2 = ExitStack()
    psA2 = ps2.enter_context(tc.tile_pool(name="psA2", bufs=3, space="PSUM"))
    psS2 = ps2.enter_context(tc.tile_pool(name="psS2", bufs=1, space="PSUM"))
    ch2 = Chunk(2, psS2)
    for o in range(NOCT):
        emit_octet(ch2, o, psA2)
    finish_chunk(ch2, psS2, "tp")
    ps2.close()

    ctx.close()


def build(nn, rr, esc, esc_q, esc_k, sig_scale, sig_bias, num_devices=N_CORES):
    big = pick_big(sig_scale)
    nc = bacc.Bacc("TRN2", target_bir_lowering=False, debug=False,
                   num_devices=num_devices)
    nch = 3
    noct = nn // 128 // 8
    io = {
        "adjt": nc.dram_tensor("adjt", [nch * noct * 128, 8 * 512], BF16,
                               kind="ExternalInput").ap(),
        "xT": nc.dram_tensor("xT", [65, nn], BF16, kind="ExternalInput").ap(),
        "xqT": nc.dram_tensor("xqT", [65, rr], BF16,
                              kind="ExternalInput").ap(),
        "wT": nc.dram_tensor("wT", [65, 64], BF16, kind="ExternalInput").ap(),
        "wqT": nc.dram_tensor("wqT", [65, 64], BF16,
                              kind="ExternalInput").ap(),
        "wkT": nc.dram_tensor("wkT", [65, 64], BF16,
                              kind="ExternalInput").ap(),
        "bigi": nc.dram_tensor("bigi", [128, 128], BF16,
                               kind="ExternalInput").ap(),
        "out": nc.dram_tensor("out", [rr, 64], F32, kind="ExternalOutput").ap(),
    }
    with tile.TileContext(nc) as tc:
        emit(tc, io, nn, rr, esc, esc_q, esc_k, sig_scale, sig_bias, big)
    nc.compile()
    return nc


def make_in_maps(inputs, nn, rr, n_cores):
    bf = ml_dtypes.bfloat16
    x = np.asarray(inputs["x"], np.float32)
    adj = np.ascontiguousarray(np.asarray(inputs["adj"], np.float32))
    W = np.asarray(inputs["W"], np.float32)
    b = np.asarray(inputs["b"], np.float32)
    Wq = np.asarray(inputs["Wq"], np.float32)
    bq = np.asarray(inputs["bq"], np.float32)
    Wk = np.asarray(inputs["Wk"], np.float32)
    bk = np.asarray(inputs["bk"], np.float32)

    att_scale = float(np.asarray(inputs["att_scale"], np.float32))
    big = pick_big(2.0 / att_scale)

    xT_ext = np.concatenate([x.T, np.ones((1, nn), np.float32)], 0).astype(bf)
    wT_ext = np.concatenate([W.T, b[None, :]], 0).astype(bf)
    wqT_ext = np.concatenate([Wq.T, bq[None, :]], 0).astype(bf)
    wkT_ext = np.concatenate([Wk.T, bk[None, :]], 0).astype(bf)
    bigI = (np.eye(128, dtype=np.float32) * big).astype(bf)

    in_maps = []
    for c in range(n_cores):
        r0 = c * rr
        slab = adj[r0:r0 + rr]                       # [1536, 12288]
        # adjt[(ch*12+o)*128+p, t*512+q] = slab[ch*512+q, (o*8+t)*128+p]
        # (per-partition contiguous lines for the octet DMAs)
        a6 = slab.reshape(3, 512, 12, 8, 128).transpose(0, 2, 4, 3, 1)
        adjt = np.ascontiguousarray(a6.reshape(3 * 12 * 128, 8 * 512)).astype(bf)
        in_maps.append({
            "adjt": adjt,
            "xT": np.ascontiguousarray(xT_ext),
            "xqT": np.ascontiguousarray(xT_ext[:, r0:r0 + rr]),
            "wT": wT_ext,
            "wqT": wqT_ext,
            "wkT": wkT_ext,
            "bigi": bigI,
        })
    return in_maps


def consts_from_inputs(inputs):
    scale = float(np.asarray(inputs["scale"], np.float32))
    scale_q = float(np.asarray(inputs["scale_q"], np.float32))
    scale_k = float(np.asarray(inputs["scale_k"], np.float32))
    att_bias = float(np.asarray(inputs["att_bias"], np.float32))
    att_scale = float(np.asarray(inputs["att_scale"], np.float32))
    esc = math.exp(scale)
    esc_q = math.exp(scale_q)
    esc_k = math.exp(scale_k)
    sig_scale = 2.0 / att_scale
    sig_bias = 2.0 / att_scale + att_bias
    return esc, esc_q, esc_k, sig_scale, sig_bias


def kernel(**inputs):
    nn, rr = N_FULL, R_FULL
    consts = consts_from_inputs(inputs)
    nc = build(nn, rr, *consts)
    in_maps = make_in_maps(inputs, nn, rr, N_CORES)
    res = bass_utils.run_bass_kernel_spmd(nc, in_maps,
                                          core_ids=list(range(N_CORES)))
    return np.concatenate([res.results[c]["out"] for c in range(N_CORES)],
                          axis=0)
